# revision 1
# baseline (speedup 1.0000x reference)
"""Child-sum TreeLSTM encoder over the deterministic 8-ary tree (N=200000,
D=256, 7 levels), distributed over 8 Trainium2 NeuronCores.

Sharding: the 512 "units" (unit j = 64 consecutive L5 nodes + their 512 L6
children + 8 L4 parents + 1 L3 grandparent) are dealt to cores so that every
core owns 8 whole superblocks (superblock = 8 units = 1 L2 node's subtree),
interleaved stride-8 so per-core work is uniform.  All parent<-child
aggregation is then core-local except the final 64 L2 nodes, which are
AllGathered (16 KB) before every core redundantly computes L1 + root.

On-chip layout is feature-major (x^T / h^T tiles [256=2x128 partitions,
nodes]), so weights are the PE-stationary operands and no on-chip transposes
are needed; the host pre-transposes x shards and post-transposes h shards.
Big matmuls run in float32r (fp32 with 12-bit mantissa, 4x faster than fp32).
"""

import numpy as np

# ---------------------------------------------------------------- constants
N = 200_000
D = 256
BR = 8
S = [0, 1, 9, 73, 585, 4681, 37449]   # level start offsets (levels 0..6)
UNITS = 512                            # total units; unit j <-> L3 node 73+j
N_CORES = 8
FULL_SLOTS = 40                        # phase-A child slots per core
T = 512                                # nodes per phase-A slot / phase-B tile
L5_PC, L4_PC, L3_PC, L2_PC = 4096, 512, 64, 8   # per-core node counts
A_COLS = FULL_SLOTS * T                # 20480
XF_COLS = L5_PC + L4_PC + L3_PC + L2_PC + 12    # 4692 (top section padded)

_STATE = {}


# ------------------------------------------------------- host-side indexing
def _units_of(core):
    return [8 * (core + 8 * k) + jj for k in range(8) for jj in range(8)]


def _index_maps():
    """Per-core global node-id arrays for each packed segment (-1 = pad)."""
    if "maps" in _STATE:
        return _STATE["maps"]
    maps = []
    for c in range(N_CORES):
        units = _units_of(c)
        idsA = np.full(A_COLS, -1, dtype=np.int64)
        for s in range(FULL_SLOTS):
            j = units[s]
            base = S[6] + T * j
            hi = min(N, base + T)
            if base < N:
                n = hi - base
                idsA[s * T:s * T + n] = np.arange(base, hi)
        idsB = np.concatenate([np.arange(S[5] + 64 * j, S[5] + 64 * (j + 1)) for j in units])
        idsC = np.concatenate([np.arange(S[4] + 8 * j, S[4] + 8 * (j + 1)) for j in units])
        idsD = np.array([S[3] + j for j in units], dtype=np.int64)
        idsE = np.array([S[2] + c + 8 * k for k in range(8)], dtype=np.int64)
        idsTop = np.concatenate([np.arange(1, 9), [0], [-1] * 7])
        maps.append(dict(A=idsA, B=idsB, C=idsC, D=idsD, E=idsE, Top=idsTop))
    _STATE["maps"] = maps
    return maps


# ---------------------------------------------------------------- program
class _NullDone(Exception):
    pass


def _build_program(single_core=False, null=False):
    import concourse.bacc as bacc
    import concourse.tile as tile
    import concourse.mybir as mybir

    F32 = mybir.dt.float32
    F32R = mybir.dt.float32r
    AF = mybir.ActivationFunctionType
    OP = mybir.AluOpType

    nc = bacc.Bacc(num_devices=1 if single_core else N_CORES)

    # ---- external I/O (per-core shapes; host packs per core)
    xA = nc.dram_tensor("xA", [D, A_COLS], F32, kind="ExternalInput").ap()
    xB = nc.dram_tensor("xB", [D, L5_PC], F32, kind="ExternalInput").ap()
    xC = nc.dram_tensor("xC", [D, L4_PC], F32, kind="ExternalInput").ap()
    xD = nc.dram_tensor("xD", [D, L3_PC], F32, kind="ExternalInput").ap()
    xE = nc.dram_tensor("xE", [D, L2_PC], F32, kind="ExternalInput").ap()
    xTop = nc.dram_tensor("xTop", [D, 16], F32, kind="ExternalInput").ap()
    Wiou = nc.dram_tensor("Wiou", [D, 3 * D], F32, kind="ExternalInput").ap()
    Uiou = nc.dram_tensor("Uiou", [D, 3 * D], F32, kind="ExternalInput").ap()
    Wf = nc.dram_tensor("Wf", [D, D], F32, kind="ExternalInput").ap()
    Uf = nc.dram_tensor("Uf", [D, D], F32, kind="ExternalInput").ap()

    hA = nc.dram_tensor("hA", [D, A_COLS], F32, kind="ExternalOutput").ap()
    hB = nc.dram_tensor("hB", [D, L5_PC], F32, kind="ExternalOutput").ap()
    hC = nc.dram_tensor("hC", [D, L4_PC], F32, kind="ExternalOutput").ap()
    hD = nc.dram_tensor("hD", [D, L3_PC], F32, kind="ExternalOutput").ap()
    hE = nc.dram_tensor("hE", [D, L2_PC], F32, kind="ExternalOutput").ap()
    hTop = nc.dram_tensor("hTop", [D, 16], F32, kind="ExternalOutput").ap()

    # ---- internal DRAM
    xf_dram = nc.dram_tensor("xf_dram", [D, XF_COLS], F32)  # x@Wf for internal nodes
    fc5_dram = nc.dram_tensor("fc5_dram", [D, 2560], F32).ap()
    ht5_dram = nc.dram_tensor("ht5_dram", [D, 2560], F32).ap()
    ag_in = nc.dram_tensor("ag_in", [D, 16], F32)           # my L2 h(8)||c(8)
    ag_out = (nc.dram_tensor("ag_out", [N_CORES, D, 16], F32).ap() if single_core
              else nc.dram_tensor("ag_out", [N_CORES, D, 16], F32, addr_space="Shared").ap())
    ag_in = ag_in.ap()
    xf_dram = xf_dram.ap()

    with tile.TileContext(nc) as tc:
        import contextlib
        ctx = contextlib.ExitStack()
        try:
          with ctx:
            wp = ctx.enter_context(tc.tile_pool(name="wts", bufs=1))
            acc = ctx.enter_context(tc.tile_pool(name="acc", bufs=1))
            xin = ctx.enter_context(tc.tile_pool(name="xin", bufs=5))
            gp = ctx.enter_context(tc.tile_pool(name="gates", bufs=6))
            tcp = ctx.enter_context(tc.tile_pool(name="tcs", bufs=3))
            hp = ctx.enter_context(tc.tile_pool(name="hc", bufs=6))
            fp = ctx.enter_context(tc.tile_pool(name="fs", bufs=4))
            bp = ctx.enter_context(tc.tile_pool(name="bht", bufs=2))
            pio = ctx.enter_context(tc.tile_pool(name="iou", bufs=6, space="PSUM"))
            pfp = ctx.enter_context(tc.tile_pool(name="fpre", bufs=1, space="PSUM"))

            if null:
                # same I/O signature + collective, trivial compute: isolates
                # dispatch/transfer overhead for timing calibration
                t0 = xin.tile([128, 2, T], F32R, tag="xk")
                nc.sync.dma_start(out=t0[:, :, :T],
                                  in_=xA[:, 0:T].rearrange("(k p) n -> p k n", p=128).bitcast(F32R))
                h0t = hp.tile([128, 2, T], F32R, tag="h")
                nc.vector.tensor_copy(out=h0t[:, 0, :], in_=t0[:, 0, :])
                nc.gpsimd.dma_start(out=hA[:, 0:T].rearrange("(k p) n -> p k n", p=128),
                                    in_=h0t[:, :, :T].bitcast(F32))
                agi = fp.tile([128, 2, 8], F32, tag="xfs")
                nc.vector.tensor_copy(out=agi[:], in_=t0[:, :, 0:8].bitcast(F32))
                for half in range(2):
                    nc.scalar.dma_start(out=ag_in[128 * half:128 * (half + 1), 0:8], in_=agi[:, half, :])
                    nc.scalar.dma_start(out=ag_in[128 * half:128 * (half + 1), 8:16], in_=agi[:, half, :])
                if single_core:
                    for rr in range(N_CORES):
                        nc.sync.dma_start(out=ag_out[rr], in_=ag_in[:].rearrange("(c p) n -> c p n", p=128))
                else:
                    nc.gpsimd.collective_compute(
                        "AllGather", mybir.AluOpType.bypass,
                        replica_groups=[list(range(N_CORES))],
                        ins=[ag_in[:]], outs=[ag_out[:]])
                g0 = acc.tile([128, 2, 8, 16], F32, tag="g")
                for k in range(2):
                    nc.sync.dma_start(out=g0[:, k, :, :],
                                      in_=ag_out[:, 128 * k:128 * (k + 1), :].rearrange("r p n -> p r n"))
                go = fp.tile([128, 2, T], F32, tag="f")
                nc.vector.tensor_copy(out=go[:, :, 0:16].rearrange("p m n -> p m n"), in_=g0[:, :, 0, :])
                nc.scalar.dma_start(out=hTop[:, 0:16].rearrange("(k p) n -> p k n", p=128), in_=go[:, :, 0:16])
                raise _NullDone()

            # ---------------- weights: [128, k(2), m, 128] stationary chunks
            wiou_sb = wp.tile([128, 2, 6, 128], F32R, tag="wiou")
            uiou_sb = wp.tile([128, 2, 6, 128], F32R, tag="uiou")
            wf_sb = wp.tile([128, 2, 2, 128], F32R, tag="wf")
            uf_sb = wp.tile([128, 2, 2, 128], F32R, tag="uf")
            for k in range(2):
                nc.sync.dma_start(out=wiou_sb[:, k, :, :],
                                  in_=Wiou[128 * k:128 * (k + 1), :].rearrange("p (m q) -> p m q", q=128).bitcast(F32R))
                nc.sync.dma_start(out=uiou_sb[:, k, :, :],
                                  in_=Uiou[128 * k:128 * (k + 1), :].rearrange("p (m q) -> p m q", q=128).bitcast(F32R))
                nc.sync.dma_start(out=wf_sb[:, k, :, :],
                                  in_=Wf[128 * k:128 * (k + 1), :].rearrange("p (m q) -> p m q", q=128).bitcast(F32R))
                nc.sync.dma_start(out=uf_sb[:, k, :, :],
                                  in_=Uf[128 * k:128 * (k + 1), :].rearrange("p (m q) -> p m q", q=128).bitcast(F32R))

            # ---------------- persistent accumulators
            xfB_sb = acc.tile([128, 2, 2560], F32, tag="xfB")
            fc4 = acc.tile([128, 2, L4_PC], F32, tag="fc4")
            ht4 = acc.tile([128, 2, L4_PC], F32R, tag="ht4")
            fc3 = acc.tile([128, 2, L3_PC], F32, tag="fc3")
            ht3 = acc.tile([128, 2, L3_PC], F32R, tag="ht3")
            fc2 = acc.tile([128, 2, L2_PC], F32, tag="fc2")
            ht2 = acc.tile([128, 2, L2_PC], F32R, tag="ht2")

            def load_xT(src, lo, n, dt=F32R):
                """DMA [D, lo:lo+n] of a DRAM tensor into a [128, 2, n] tile."""
                t = xin.tile([128, 2, T], dt, tag="xk")
                s = src[:, lo:lo + n].rearrange("(k p) n -> p k n", p=128)
                nc.sync.dma_start(out=t[:, :, :n], in_=s.bitcast(dt) if dt != F32 else s)
                return t

            # ---------------- phase A0: xf = x @ Wf for all internal nodes
            def xf_tile(src, src_lo, n, dst_lo, sb_dst=None):
                xk = load_xT(src, src_lo, n)
                ps = pfp.tile([128, 2, T], F32, tag="fpre")
                for m in range(2):
                    for k in range(2):
                        nc.tensor.matmul(ps[:, m, :n], wf_sb[:, k, m, :], xk[:, k, :n],
                                         start=(k == 0), stop=(k == 1))
                if sb_dst is not None:
                    nc.scalar.activation(sb_dst, ps[:, :, :n], AF.Identity)
                    return
                xf_sb = fp.tile([128, 2, T], F32, tag="f")
                nc.scalar.activation(xf_sb[:, :, :n], ps[:, :, :n], AF.Identity)
                nc.scalar.dma_start(out=xf_dram[:, dst_lo:dst_lo + n].rearrange("(k p) n -> p k n", p=128),
                                    in_=xf_sb[:, :, :n])

            for i in range(5):   # phase-A xf (internal L5 units) stays in SBUF
                xf_tile(xB, i * T, T, 0, sb_dst=xfB_sb[:, :, i * T:(i + 1) * T])
            for i in range(5, 8):
                xf_tile(xB, i * T, T, i * T)
            xf_tile(xC, 0, T, L5_PC)
            xf_tile(xD, 0, L3_PC, L5_PC + L4_PC)
            xf_tile(xE, 0, L2_PC, L5_PC + L4_PC + L3_PC)
            xf_tile(xTop, 0, 12, L5_PC + L4_PC + L3_PC + L2_PC)

            # ---------------- generic node-tile processing
            def process_head(xk, n, ht_rhs, fc_in, h_out, out_lo):
                """Gates + c,h for n nodes; returns (h, c) tiles."""
                sio = []
                c = hp.tile([128, 2, T], F32, tag="c")
                h = hp.tile([128, 2, T], F32R, tag="h")
                for half in range(2):
                    so = gp.tile([128, 2, T], F32, tag="sio")
                    tu = gp.tile([128, T], F32, tag="tu")
                    for g, m in enumerate((half, 2 + half, 4 + half)):  # i, o, u chunks
                        ps = pio.tile([128, T], F32, tag="ps1")
                        last = ht_rhs is None
                        for k in range(2):
                            nc.tensor.matmul(ps[:, :n], wiou_sb[:, k, m, :], xk[:, k, :n],
                                             start=(k == 0), stop=(last and k == 1))
                        if ht_rhs is not None:
                            for k in range(2):
                                nc.tensor.matmul(ps[:, :n], uiou_sb[:, k, m, :], ht_rhs[:, k, :],
                                                 start=False, stop=(k == 1))
                        if g < 2:
                            nc.scalar.activation(so[:, g, :n], ps[:, :n], AF.Sigmoid)
                        else:
                            nc.scalar.activation(tu[:, :n], ps[:, :n], AF.Tanh)
                    nc.vector.tensor_tensor(out=c[:, half, :n], in0=so[:, 0, :n], in1=tu[:, :n], op=OP.mult)
                    sio.append(so)
                if fc_in is not None:
                    nc.vector.tensor_tensor(out=c[:, :, :n], in0=c[:, :, :n], in1=fc_in[:, :, :], op=OP.add)
                tc_ = tcp.tile([128, 2, T], F32, tag="tc")
                nc.scalar.activation(tc_[:, :, :n], c[:, :, :n], AF.Tanh)
                for half in range(2):
                    nc.gpsimd.tensor_tensor(out=h[:, half, :n], in0=sio[half][:, 1, :n], in1=tc_[:, half, :n], op=OP.mult)
                nc.sync.dma_start(out=h_out[:, out_lo:out_lo + n].rearrange("(k p) n -> p k n", p=128),
                                   in_=h[:, :, :n].bitcast(F32))
                return h, c

            def process_tail(h, c, n, ngrp, fc_dst, ht_dst, dst_lo,
                             xf_lo=0, xf_sb_slice=None, dram_lo=None):
                """Edges toward parents: f, fc, h-tilde (groups of 8).
                If dram_lo is not None, emit to fc5/ht5 DRAM at that column;
                else write fc_dst/ht_dst[:, :, dst_lo:+ngrp] (SBUF)."""
                hs = [h[:, 0, :], h[:, 1, :]]
                fpre = pfp.tile([128, 2, T], F32, tag="fpre")
                for m in range(2):
                    for k in range(2):
                        nc.tensor.matmul(fpre[:, m, :n], uf_sb[:, k, m, :], hs[k][:, :n],
                                         start=(k == 0), stop=(k == 1))
                if xf_sb_slice is not None:
                    xfs_ap = xf_sb_slice
                else:
                    xfs = fp.tile([128, 2, T // 8], F32, tag="xfs")
                    nc.sync.dma_start(out=xfs[:, :, :ngrp],
                                      in_=xf_dram[:, xf_lo:xf_lo + ngrp].rearrange("(k p) n -> p k n", p=128))
                    xfs_ap = xfs[:, :, :ngrp]
                nc.vector.tensor_tensor(
                    out=fpre[:, :, :n].rearrange("p m (g e) -> p m g e", e=8),
                    in0=fpre[:, :, :n].rearrange("p m (g e) -> p m g e", e=8),
                    in1=xfs_ap.rearrange("p m (g o) -> p m g o", o=1).broadcast_to([128, 2, ngrp, 8]),
                    op=OP.add)
                f = fp.tile([128, 2, T], F32, tag="f")
                nc.scalar.activation(f[:, :, :n], fpre[:, :, :n], AF.Sigmoid)
                nc.gpsimd.tensor_tensor(out=f[:, :, :n], in0=f[:, :, :n], in1=c[:, :, :n], op=OP.mult)
                if dram_lo is not None:
                    fc_dst = fp.tile([128, 2, T // 8], F32, tag="fcout")
                    ht_dst = fp.tile([128, 2, T // 8], F32, tag="htout")
                    dst_lo = 0
                for half in range(2):
                    nc.vector.tensor_reduce(
                        out=fc_dst[:, half, dst_lo:dst_lo + ngrp],
                        in_=f[:, half, :n].rearrange("p (g e) -> p g e", e=8),
                        axis=mybir.AxisListType.X, op=OP.add)
                    with nc.allow_low_precision("h-tilde feeds fp32r matmul"):
                        nc.vector.tensor_reduce(
                            out=ht_dst[:, half, dst_lo:dst_lo + ngrp],
                            in_=h[:, half, :n].bitcast(F32).rearrange("p (g e) -> p g e", e=8),
                            axis=mybir.AxisListType.X, op=OP.add)
                if dram_lo is not None:
                    nc.sync.dma_start(out=fc5_dram[:, dram_lo:dram_lo + ngrp].rearrange("(k p) n -> p k n", p=128),
                                      in_=fc_dst[:, :, :ngrp])
                    nc.sync.dma_start(out=ht5_dram[:, dram_lo:dram_lo + ngrp].rearrange("(k p) n -> p k n", p=128),
                                      in_=ht_dst[:, :, :ngrp])

            # ---------------- phase A: 40 slots of 512 leaf children
            G = 4
            for grp in range(FULL_SLOTS // G):
                hcs = []
                for s in range(grp * G, grp * G + G):
                    xk = load_xT(xA, s * T, T)
                    hcs.append(process_head(xk, T, None, None, hA, s * T))
                for i, s in enumerate(range(grp * G, grp * G + G)):
                    h_, c_ = hcs[i]
                    process_tail(h_, c_, T, 64, None, None, 0,
                                 xf_sb_slice=xfB_sb[:, :, s * 64:(s + 1) * 64],
                                 dram_lo=s * 64)

            # ---------------- phase B: 8 tiles of 512 L5 nodes
            for grp in range(2):
                hcs = []
                for t in range(grp * 4, grp * 4 + 4):
                    xk = load_xT(xB, t * T, T)
                    internal = t < 5   # cols [0,2560) of ht5/fc5 are written
                    if internal:
                        ht_in = bp.tile([128, 2, T], F32R, tag="htin")
                        nc.sync.dma_start(out=ht_in[:],
                                          in_=ht5_dram[:, t * T:(t + 1) * T].rearrange("(k p) n -> p k n", p=128).bitcast(F32R))
                        fc_in = bp.tile([128, 2, T], F32, tag="fcin")
                        nc.sync.dma_start(out=fc_in[:],
                                          in_=fc5_dram[:, t * T:(t + 1) * T].rearrange("(k p) n -> p k n", p=128))
                        ht_ap, fc_ap = ht_in[:, :, :], fc_in[:, :, :]
                    else:
                        ht_ap, fc_ap = None, None
                    xk = xk
                    hcs.append(process_head(xk, T, ht_ap, fc_ap, hB, t * T))
                for i, t in enumerate(range(grp * 4, grp * 4 + 4)):
                    h_, c_ = hcs[i]
                    process_tail(h_, c_, T, 64, fc4, ht4, t * 64,
                                 xf_lo=L5_PC + t * 64)

            # ---------------- phase C: 512 L4 nodes
            xk = load_xT(xC, 0, T)
            h_, c_ = process_head(xk, T, ht4[:, :, :], fc4[:, :, :], hC, 0)
            process_tail(h_, c_, T, 64, fc3, ht3, 0, xf_lo=L5_PC + L4_PC)

            # ---------------- phase D: 64 L3 nodes
            xk = load_xT(xD, 0, L3_PC)
            h_, c_ = process_head(xk, L3_PC, ht3[:, :, :], fc3[:, :, :], hD, 0)
            process_tail(h_, c_, L3_PC, 8, fc2, ht2, 0, xf_lo=L5_PC + L4_PC + L3_PC)

            # ---------------- phase E: 8 L2 nodes (no local emission)
            xk = load_xT(xE, 0, L2_PC)
            h2t, c2t = process_head(xk, L2_PC, ht2[:, :, :], fc2[:, :, :], hE, 0)
            hs2 = [h2t[:, 0, :], h2t[:, 1, :]]
            cs2 = [c2t[:, 0, :], c2t[:, 1, :]]
            # pack h||c into ag_in and AllGather
            for half in range(2):
                nc.scalar.dma_start(out=ag_in[128 * half:128 * (half + 1), 0:8], in_=hs2[half][:, :8].bitcast(F32))
                nc.scalar.dma_start(out=ag_in[128 * half:128 * (half + 1), 8:16], in_=cs2[half][:, :8])
            if single_core:
                # timing-equivalent stand-in for the 16KB AllGather
                for rr in range(N_CORES):
                    nc.sync.dma_start(out=ag_out[rr], in_=ag_in[:].rearrange("(c p) n -> c p n", p=128))
            else:
                nc.gpsimd.collective_compute(
                    "AllGather", mybir.AluOpType.bypass,
                    replica_groups=[list(range(N_CORES))],
                    ins=[ag_in[:]], outs=[ag_out[:]])

            # gathered: [r, (kc p), n] -> SBUF [p, kc, r, n]
            g = acc.tile([128, 2, 8, 16], F32, tag="g")
            for k in range(2):
                nc.sync.dma_start(out=g[:, k, :, :],
                                  in_=ag_out[:, 128 * k:128 * (k + 1), :].rearrange("r p n -> p r n"))
            gh = g[:, :, :, 0:8].rearrange("p c r m -> p c m r")   # [128,2,8(m),8(r)]
            gc = g[:, :, :, 8:16].rearrange("p c r m -> p c m r")
            ghr = gh.bitcast(F32R)

            # ---------------- phase F: L1 nodes 1..8 from gathered L2 children
            xf_top = L5_PC + L4_PC + L3_PC + L2_PC
            ht1 = acc.tile([128, 2, 8], F32R, tag="ht1")
            fc1 = acc.tile([128, 2, 8], F32, tag="fc1")
            fpre = pfp.tile([128, 2, T], F32, tag="fpre")
            for m in range(2):
                for k in range(2):
                    nc.tensor.matmul(fpre[:, m, :64], uf_sb[:, k, m, :],
                                     ghr[:, k, :, :], start=(k == 0), stop=(k == 1))
            xfs = fp.tile([128, 2, T // 8], F32, tag="xfs")
            nc.sync.dma_start(out=xfs[:, :, :8],
                              in_=xf_dram[:, xf_top:xf_top + 8].rearrange("(k p) n -> p k n", p=128))
            nc.vector.tensor_tensor(
                out=fpre[:, :, :64].rearrange("p m (g e) -> p m g e", e=8),
                in0=fpre[:, :, :64].rearrange("p m (g e) -> p m g e", e=8),
                in1=xfs[:, :, :8].rearrange("p m (g o) -> p m g o", o=1).broadcast_to([128, 2, 8, 8]),
                op=OP.add)
            f = fp.tile([128, 2, T], F32, tag="f")
            nc.scalar.activation(f[:, :, :64], fpre[:, :, :64], AF.Sigmoid)
            for half in range(2):
                nc.vector.tensor_tensor(out=f[:, half, :64].rearrange("p (m r) -> p m r", r=8),
                                        in0=f[:, half, :64].rearrange("p (m r) -> p m r", r=8),
                                        in1=gc[:, half, :, :], op=OP.mult)
                nc.vector.tensor_reduce(out=fc1[:, half, :],
                                        in_=f[:, half, :64].rearrange("p (g e) -> p g e", e=8),
                                        axis=mybir.AxisListType.X, op=OP.add)
                with nc.allow_low_precision("h-tilde feeds fp32r matmul"):
                    nc.vector.tensor_reduce(out=ht1[:, half, :],
                                            in_=gh[:, half, :, :],
                                            axis=mybir.AxisListType.X, op=OP.add)
            xk = load_xT(xTop, 0, 9)
            h1t, c1t = process_head(xk, 8, ht1[:, :, :], fc1[:, :, :], hTop, 0)
            hs1 = [h1t[:, 0, :], h1t[:, 1, :]]
            cs1 = [c1t[:, 0, :], c1t[:, 1, :]]

            # ---------------- phase G: root (children = L1 nodes, local)
            ht0 = acc.tile([128, 2, 4], F32R, tag="ht0")
            fc0 = acc.tile([128, 2, 4], F32, tag="fc0")
            nc.vector.tensor_copy(out=ht0[:], in_=nc.const_aps.tensor(0.0, [128, 1]).broadcast_to([128, 2, 4]))
            nc.vector.tensor_copy(out=fc0[:], in_=nc.const_aps.tensor(0.0, [128, 1]).broadcast_to([128, 2, 4]))
            fpre = pfp.tile([128, 2, T], F32, tag="fpre")
            for m in range(2):
                for k in range(2):
                    nc.tensor.matmul(fpre[:, m, :8], uf_sb[:, k, m, :], hs1[k][:, :8],
                                     start=(k == 0), stop=(k == 1))
            xfs = fp.tile([128, 2, T // 8], F32, tag="xfs")
            nc.sync.dma_start(out=xfs[:, :, :1],
                              in_=xf_dram[:, xf_top + 8:xf_top + 9].rearrange("(k p) n -> p k n", p=128))
            nc.vector.tensor_tensor(
                out=fpre[:, :, :8].rearrange("p m (g e) -> p m g e", e=8),
                in0=fpre[:, :, :8].rearrange("p m (g e) -> p m g e", e=8),
                in1=xfs[:, :, :1].rearrange("p m (g o) -> p m g o", o=1).broadcast_to([128, 2, 1, 8]),
                op=OP.add)
            f = fp.tile([128, 2, T], F32, tag="f")
            nc.scalar.activation(f[:, :, :8], fpre[:, :, :8], AF.Sigmoid)
            for half in range(2):
                nc.vector.tensor_tensor(out=f[:, half, :8], in0=f[:, half, :8],
                                        in1=cs1[half][:, :8], op=OP.mult)
                nc.vector.tensor_reduce(out=fc0[:, half, 0:1],
                                        in_=f[:, half, :8].rearrange("p (g e) -> p g e", e=8),
                                        axis=mybir.AxisListType.X, op=OP.add)
                with nc.allow_low_precision("h-tilde feeds fp32r matmul"):
                    nc.vector.tensor_reduce(out=ht0[:, half, 0:1],
                                            in_=hs1[half][:, :8].bitcast(F32).rearrange("p (g e) -> p g e", e=8),
                                            axis=mybir.AxisListType.X, op=OP.add)
            xk_root = load_xT(xTop, 8, 4)
            process_head(xk_root, 4, ht0[:, :, :], fc0[:, :, :], hTop, 8)

        except _NullDone:
            pass

    nc.compile()
    return nc


def _get_nc(variant="main"):
    key = "nc_" + variant
    if key not in _STATE:
        _STATE[key] = _build_program(null=(variant == "null"))
    return _STATE[key]


# ---------------------------------------------------------------- host glue
def _pack_inputs(x, W_iou, U_iou, W_f, U_f):
    maps = _index_maps()
    x = np.asarray(x, dtype=np.float32)
    x_pad = np.vstack([x, np.zeros((1, D), np.float32)])  # row N = zeros (for -1 ids)
    shared = {
        "Wiou": np.ascontiguousarray(np.asarray(W_iou, np.float32)),
        "Uiou": np.ascontiguousarray(np.asarray(U_iou, np.float32)),
        "Wf": np.ascontiguousarray(np.asarray(W_f, np.float32)),
        "Uf": np.ascontiguousarray(np.asarray(U_f, np.float32)),
    }
    in_maps = []
    for c in range(N_CORES):
        m = maps[c]
        d = dict(shared)
        for name in ("A", "B", "C", "D", "E", "Top"):
            ids = m[name]
            d["x" + name] = np.ascontiguousarray(x_pad[ids].T)
        in_maps.append(d)
    return in_maps


def _unpack_outputs(results):
    maps = _index_maps()
    out = np.empty((N, D), dtype=np.float32)
    for c in range(N_CORES):
        m = maps[c]
        for name, key in (("A", "hA"), ("B", "hB"), ("C", "hC"), ("D", "hD"), ("E", "hE")):
            ids = m[name]
            valid = ids >= 0
            out[ids[valid]] = results[c][key].T[valid]
    top = results[0]["hTop"].T          # rows: L1 nodes 1..8 then root 0
    out[1:9] = top[0:8]
    out[0] = top[8]
    return out


# ---------------------------------------------------------------- jitted runner
def _get_runner(variant="main"):
    """Jit the SPMD executable once; returns run(in_maps, zero_outs=None)."""
    rkey = "runner_" + variant
    if rkey in _STATE:
        return _STATE[rkey]
    import jax
    from jax.sharding import Mesh, PartitionSpec, NamedSharding
    from jax.experimental.shard_map import shard_map
    import concourse.mybir as mybir
    from concourse import bass2jax

    nc = _get_nc(variant)
    bass2jax.install_neuronx_cc_hook()

    partition_name = nc.partition_id_tensor.name if nc.partition_id_tensor else None
    in_names, out_names, out_avals = [], [], []
    for alloc in nc.m.functions[0].allocations:
        if not isinstance(alloc, mybir.MemoryLocationSet):
            continue
        name = alloc.memorylocations[0].name
        if alloc.kind == "ExternalInput":
            if name != partition_name:
                in_names.append(name)
        elif alloc.kind == "ExternalOutput":
            out_names.append(name)
            out_avals.append(jax.core.ShapedArray(tuple(alloc.tensor_shape),
                                                  mybir.dt.np(alloc.dtype)))
    n_params = len(in_names)
    n_outs = len(out_names)
    all_names = in_names + out_names
    if partition_name is not None:
        all_names = all_names + [partition_name]

    def _body(*args):
        operands = list(args)
        if partition_name is not None:
            operands.append(bass2jax.partition_id_tensor())
        outs = bass2jax._bass_exec_p.bind(
            *operands,
            out_avals=tuple(out_avals),
            in_names=tuple(all_names),
            out_names=tuple(out_names),
            lowering_input_output_aliases=(),
            sim_require_finite=True,
            sim_require_nnan=True,
            nc=nc,
        )
        return tuple(outs)

    devices = jax.devices()[:N_CORES]
    mesh = Mesh(np.asarray(devices), ("core",))
    sharding = NamedSharding(mesh, PartitionSpec("core"))
    donate = tuple(range(n_params, n_params + n_outs))
    sharded = jax.jit(
        shard_map(_body, mesh=mesh,
                  in_specs=(PartitionSpec("core"),) * (n_params + n_outs),
                  out_specs=(PartitionSpec("core"),) * n_outs,
                  check_rep=False),
        donate_argnums=donate, keep_unused=True)

    def make_zero_outs():
        return [jax.device_put(
            np.zeros((N_CORES * a.shape[0], *a.shape[1:]), a.dtype), sharding)
            for a in out_avals]

    def put_inputs(in_maps):
        return [jax.device_put(
            np.concatenate([np.asarray(in_maps[c][n]) for c in range(N_CORES)], axis=0),
            sharding) for n in in_names]

    def run(dev_inputs, zero_outs):
        out_arrs = sharded(*dev_inputs, *zero_outs)
        jax.block_until_ready(out_arrs)
        return out_arrs

    def to_results(out_arrs):
        return [
            {name: np.asarray(out_arrs[i]).reshape(N_CORES, *out_avals[i].shape)[c]
             for i, name in enumerate(out_names)}
            for c in range(N_CORES)
        ]

    r = dict(run=run, put_inputs=put_inputs, make_zero_outs=make_zero_outs,
             to_results=to_results, out_names=out_names, in_names=in_names)
    _STATE[rkey] = r
    return r


def _run(in_maps):
    r = _get_runner()
    dev_inputs = r["put_inputs"](in_maps)
    out_arrs = r["run"](dev_inputs, r["make_zero_outs"]())
    return r["to_results"](out_arrs)


def kernel(x, W_iou, U_iou, b_iou, W_f, U_f, b_f, parent_idx, level, num_levels):
    b_iou = np.asarray(b_iou)
    b_f = np.asarray(b_f)
    assert not b_iou.any() and not b_f.any(), "kernel assumes zero biases"
    parent_idx = np.asarray(parent_idx)
    expect_parent = (np.arange(1, N, dtype=np.int64) - 1) // BR
    assert parent_idx.shape == (N - 1,) and np.array_equal(parent_idx, expect_parent), \
        "kernel is specialized to the deterministic 8-ary tree parent(i)=(i-1)//8"
    in_maps = _pack_inputs(x, W_iou, U_iou, W_f, U_f)
    results = _run(in_maps)
    return _unpack_outputs(results)



# revision 5
# speedup vs baseline: 91.2654x; 91.2654x over previous
"""Child-sum TreeLSTM encoder over the deterministic 8-ary tree (N=200000,
D=256, 7 levels), distributed over 8 Trainium2 NeuronCores.

Sharding: the 512 "units" (unit j = 64 consecutive L5 nodes + their 512 L6
children + 8 L4 parents + 1 L3 grandparent) are dealt to cores so that every
core owns 8 whole superblocks (superblock = 8 units = 1 L2 node's subtree),
interleaved stride-8 so per-core work is uniform.  All parent<-child
aggregation is core-local up to and including the 64 L2 nodes (levels 6..2,
199,927 of the 200,000 nodes); the 9 top nodes (8 L1 + root) are finished on
the host from the device-computed L2 (h, c) states, so the device program
has no collectives and the 8 cores run fully independently.

All per-core external I/O is packed into a single input tensor ``xw``
(x sections A|B|C|D|E followed by Wiou|Uiou|Wf|Uf) and a single output
tensor ``hAll`` (h sections A|B|C|D|E followed by c of the L2 nodes) to
minimize per-dispatch buffer-handling overhead through the axon tunnel.

On-chip layout is feature-major (x^T / h^T tiles [256=2x128 partitions,
nodes]), so weights are the PE-stationary operands and no on-chip transposes
are needed; the host pre-transposes x shards and post-transposes h shards.
Big matmuls run in float32r (fp32 with 12-bit mantissa, 4x faster than fp32).
"""

import numpy as np

# ---------------------------------------------------------------- constants
N = 200_000
D = 256
BR = 8
S = [0, 1, 9, 73, 585, 4681, 37449]   # level start offsets (levels 0..6)
UNITS = 512                            # total units; unit j <-> L3 node 73+j
N_CORES = 8
FULL_SLOTS = 40                        # phase-A child slots per core
T = 512                                # nodes per phase-A slot / phase-B tile
L5_PC, L4_PC, L3_PC, L2_PC = 4096, 512, 64, 8   # per-core node counts
A_COLS = FULL_SLOTS * T                # 20480

# packed input xw column offsets: x sections then weights
OFF_A = 0
OFF_B = OFF_A + A_COLS                 # 20480
OFF_C = OFF_B + L5_PC                  # 24576
OFF_D = OFF_C + L4_PC                  # 25088
OFF_E = OFF_D + L3_PC                  # 25152
X_COLS = OFF_E + L2_PC                 # 25160
OFF_WIOU = X_COLS                      # 25160
OFF_UIOU = OFF_WIOU + 3 * D            # 25928
OFF_WF = OFF_UIOU + 3 * D              # 26696
OFF_UF = OFF_WF + D                    # 26952
XW_COLS = OFF_UF + D                   # 27208

# packed output hAll: h sections A|B|C|D|E then c of the 8 L2 nodes
OFF_CE = X_COLS                        # 25160
H_COLS = OFF_CE + L2_PC                # 25168

# internal xf scratch (x@Wf for parents): C | D | E sections only
# (phase-A parents' xf stays in SBUF; childless L5 nodes need no xf)
XF_C = 0
XF_D = XF_C + L4_PC                    # 512
XF_E = XF_D + L3_PC                    # 576
XF_COLS = XF_E + L2_PC                 # 584

_STATE = {}


# ------------------------------------------------------- host-side indexing
def _units_of(core):
    return [8 * (core + 8 * k) + jj for k in range(8) for jj in range(8)]


def _index_maps():
    """Per-core global node-id arrays for each packed segment (-1 = pad)."""
    if "maps" in _STATE:
        return _STATE["maps"]
    maps = []
    for c in range(N_CORES):
        units = _units_of(c)
        idsA = np.full(A_COLS, -1, dtype=np.int64)
        for s in range(FULL_SLOTS):
            j = units[s]
            base = S[6] + T * j
            hi = min(N, base + T)
            if base < N:
                n = hi - base
                idsA[s * T:s * T + n] = np.arange(base, hi)
        idsB = np.concatenate([np.arange(S[5] + 64 * j, S[5] + 64 * (j + 1)) for j in units])
        idsC = np.concatenate([np.arange(S[4] + 8 * j, S[4] + 8 * (j + 1)) for j in units])
        idsD = np.array([S[3] + j for j in units], dtype=np.int64)
        idsE = np.array([S[2] + c + 8 * k for k in range(8)], dtype=np.int64)
        maps.append(dict(A=idsA, B=idsB, C=idsC, D=idsD, E=idsE))
    _STATE["maps"] = maps
    return maps


# ---------------------------------------------------------------- program
def _build_program(null=False):
    import concourse.bacc as bacc
    import concourse.tile as tile
    import concourse.mybir as mybir

    F32 = mybir.dt.float32
    F32R = mybir.dt.float32r
    AF = mybir.ActivationFunctionType
    OP = mybir.AluOpType

    nc = bacc.Bacc(num_devices=N_CORES)

    # ---- external I/O (per-core shapes; host packs per core)
    xw = nc.dram_tensor("xw", [D, XW_COLS], F32, kind="ExternalInput").ap()
    hAll = nc.dram_tensor("hAll", [D, H_COLS], F32, kind="ExternalOutput").ap()

    if null:
        # same I/O signature, trivial compute: isolates dispatch overhead
        # for timing calibration
        with tile.TileContext(nc) as tc:
            with tc.tile_pool(name="p", bufs=1) as p:
                t0 = p.tile([128, 2, T], F32, tag="xk")
                nc.sync.dma_start(out=t0[:, :, :T],
                                  in_=xw[:, 0:T].rearrange("(k p) n -> p k n", p=128))
                h0t = p.tile([128, 2, T], F32, tag="h")
                nc.vector.tensor_copy(out=h0t[:], in_=t0[:])
                nc.gpsimd.dma_start(out=hAll[:, 0:T].rearrange("(k p) n -> p k n", p=128),
                                    in_=h0t[:, :, :T])
        nc.compile()
        return nc

    # ---- internal DRAM
    xf_dram = nc.dram_tensor("xf_dram", [D, XF_COLS], F32).ap()
    fc5_dram = nc.dram_tensor("fc5_dram", [D, 2560], F32).ap()
    ht5_dram = nc.dram_tensor("ht5_dram", [D, 2560], F32).ap()

    with tile.TileContext(nc) as tc:
        import contextlib
        with contextlib.ExitStack() as ctx:
            wp = ctx.enter_context(tc.tile_pool(name="wts", bufs=1))
            acc = ctx.enter_context(tc.tile_pool(name="acc", bufs=1))
            xin = ctx.enter_context(tc.tile_pool(name="xin", bufs=5))
            gp = ctx.enter_context(tc.tile_pool(name="gates", bufs=6))
            tcp = ctx.enter_context(tc.tile_pool(name="tcs", bufs=3))
            hp = ctx.enter_context(tc.tile_pool(name="hc", bufs=6))
            fp = ctx.enter_context(tc.tile_pool(name="fs", bufs=4))
            bp = ctx.enter_context(tc.tile_pool(name="bht", bufs=2))
            pio = ctx.enter_context(tc.tile_pool(name="iou", bufs=6, space="PSUM"))
            pfp = ctx.enter_context(tc.tile_pool(name="fpre", bufs=1, space="PSUM"))

            # ---------------- weights: [128, k(2), m, 128] stationary chunks
            wiou_sb = wp.tile([128, 2, 6, 128], F32R, tag="wiou")
            uiou_sb = wp.tile([128, 2, 6, 128], F32R, tag="uiou")
            wf_sb = wp.tile([128, 2, 2, 128], F32R, tag="wf")
            uf_sb = wp.tile([128, 2, 2, 128], F32R, tag="uf")
            for k in range(2):
                rows = slice(128 * k, 128 * (k + 1))
                nc.sync.dma_start(out=wiou_sb[:, k, :, :],
                                  in_=xw[rows, OFF_WIOU:OFF_WIOU + 3 * D].rearrange("p (m q) -> p m q", q=128).bitcast(F32R))
                nc.sync.dma_start(out=uiou_sb[:, k, :, :],
                                  in_=xw[rows, OFF_UIOU:OFF_UIOU + 3 * D].rearrange("p (m q) -> p m q", q=128).bitcast(F32R))
                nc.sync.dma_start(out=wf_sb[:, k, :, :],
                                  in_=xw[rows, OFF_WF:OFF_WF + D].rearrange("p (m q) -> p m q", q=128).bitcast(F32R))
                nc.sync.dma_start(out=uf_sb[:, k, :, :],
                                  in_=xw[rows, OFF_UF:OFF_UF + D].rearrange("p (m q) -> p m q", q=128).bitcast(F32R))

            # ---------------- persistent accumulators
            xfB_sb = acc.tile([128, 2, 2560], F32, tag="xfB")
            fc4 = acc.tile([128, 2, L4_PC], F32, tag="fc4")
            ht4 = acc.tile([128, 2, L4_PC], F32R, tag="ht4")
            fc3 = acc.tile([128, 2, L3_PC], F32, tag="fc3")
            ht3 = acc.tile([128, 2, L3_PC], F32R, tag="ht3")
            fc2 = acc.tile([128, 2, L2_PC], F32, tag="fc2")
            ht2 = acc.tile([128, 2, L2_PC], F32R, tag="ht2")

            def load_xT(lo, n, dt=F32R):
                """DMA xw[:, lo:lo+n] into a [128, 2, n] tile (feature-major)."""
                t = xin.tile([128, 2, T], dt, tag="xk")
                s = xw[:, lo:lo + n].rearrange("(k p) n -> p k n", p=128)
                nc.sync.dma_start(out=t[:, :, :n], in_=s.bitcast(dt) if dt != F32 else s)
                return t

            # ---------------- phase A0: xf = x @ Wf for parent nodes
            def xf_tile(src_lo, n, dst_lo=None, sb_dst=None):
                xk = load_xT(src_lo, n)
                ps = pfp.tile([128, 2, T], F32, tag="fpre")
                for m in range(2):
                    for k in range(2):
                        nc.tensor.matmul(ps[:, m, :n], wf_sb[:, k, m, :], xk[:, k, :n],
                                         start=(k == 0), stop=(k == 1))
                if sb_dst is not None:
                    nc.scalar.activation(sb_dst, ps[:, :, :n], AF.Identity)
                    return
                xf_sb = fp.tile([128, 2, T], F32, tag="f")
                nc.scalar.activation(xf_sb[:, :, :n], ps[:, :, :n], AF.Identity)
                nc.scalar.dma_start(out=xf_dram[:, dst_lo:dst_lo + n].rearrange("(k p) n -> p k n", p=128),
                                    in_=xf_sb[:, :, :n])

            for i in range(5):   # phase-A parents' xf (internal L5 units) stays in SBUF
                xf_tile(OFF_B + i * T, T, sb_dst=xfB_sb[:, :, i * T:(i + 1) * T])
            xf_tile(OFF_C, T, XF_C)
            xf_tile(OFF_D, L3_PC, XF_D)
            xf_tile(OFF_E, L2_PC, XF_E)

            # ---------------- generic node-tile processing
            def process_head(xk, n, ht_rhs, fc_in, out_lo):
                """Gates + c,h for n nodes; returns (h, c) tiles."""
                sio = []
                c = hp.tile([128, 2, T], F32, tag="c")
                h = hp.tile([128, 2, T], F32R, tag="h")
                for half in range(2):
                    so = gp.tile([128, 2, T], F32, tag="sio")
                    tu = gp.tile([128, T], F32, tag="tu")
                    for g, m in enumerate((half, 2 + half, 4 + half)):  # i, o, u chunks
                        ps = pio.tile([128, T], F32, tag="ps1")
                        last = ht_rhs is None
                        for k in range(2):
                            nc.tensor.matmul(ps[:, :n], wiou_sb[:, k, m, :], xk[:, k, :n],
                                             start=(k == 0), stop=(last and k == 1))
                        if ht_rhs is not None:
                            for k in range(2):
                                nc.tensor.matmul(ps[:, :n], uiou_sb[:, k, m, :], ht_rhs[:, k, :],
                                                 start=False, stop=(k == 1))
                        if g < 2:
                            nc.scalar.activation(so[:, g, :n], ps[:, :n], AF.Sigmoid)
                        else:
                            nc.scalar.activation(tu[:, :n], ps[:, :n], AF.Tanh)
                    nc.vector.tensor_tensor(out=c[:, half, :n], in0=so[:, 0, :n], in1=tu[:, :n], op=OP.mult)
                    sio.append(so)
                if fc_in is not None:
                    nc.vector.tensor_tensor(out=c[:, :, :n], in0=c[:, :, :n], in1=fc_in[:, :, :], op=OP.add)
                tc_ = tcp.tile([128, 2, T], F32, tag="tc")
                nc.scalar.activation(tc_[:, :, :n], c[:, :, :n], AF.Tanh)
                for half in range(2):
                    nc.gpsimd.tensor_tensor(out=h[:, half, :n], in0=sio[half][:, 1, :n], in1=tc_[:, half, :n], op=OP.mult)
                nc.sync.dma_start(out=hAll[:, out_lo:out_lo + n].rearrange("(k p) n -> p k n", p=128),
                                  in_=h[:, :, :n].bitcast(F32))
                return h, c

            def process_tail(h, c, n, ngrp, fc_dst, ht_dst, dst_lo,
                             xf_lo=0, xf_sb_slice=None, dram_lo=None):
                """Edges toward parents: f, fc, h-tilde (groups of 8).
                If dram_lo is not None, emit to fc5/ht5 DRAM at that column;
                else write fc_dst/ht_dst[:, :, dst_lo:+ngrp] (SBUF)."""
                hs = [h[:, 0, :], h[:, 1, :]]
                fpre = pfp.tile([128, 2, T], F32, tag="fpre")
                for m in range(2):
                    for k in range(2):
                        nc.tensor.matmul(fpre[:, m, :n], uf_sb[:, k, m, :], hs[k][:, :n],
                                         start=(k == 0), stop=(k == 1))
                if xf_sb_slice is not None:
                    xfs_ap = xf_sb_slice
                else:
                    xfs = fp.tile([128, 2, T // 8], F32, tag="xfs")
                    nc.sync.dma_start(out=xfs[:, :, :ngrp],
                                      in_=xf_dram[:, xf_lo:xf_lo + ngrp].rearrange("(k p) n -> p k n", p=128))
                    xfs_ap = xfs[:, :, :ngrp]
                nc.vector.tensor_tensor(
                    out=fpre[:, :, :n].rearrange("p m (g e) -> p m g e", e=8),
                    in0=fpre[:, :, :n].rearrange("p m (g e) -> p m g e", e=8),
                    in1=xfs_ap.rearrange("p m (g o) -> p m g o", o=1).broadcast_to([128, 2, ngrp, 8]),
                    op=OP.add)
                f = fp.tile([128, 2, T], F32, tag="f")
                nc.scalar.activation(f[:, :, :n], fpre[:, :, :n], AF.Sigmoid)
                nc.gpsimd.tensor_tensor(out=f[:, :, :n], in0=f[:, :, :n], in1=c[:, :, :n], op=OP.mult)
                if dram_lo is not None:
                    fc_dst = fp.tile([128, 2, T // 8], F32, tag="fcout")
                    ht_dst = fp.tile([128, 2, T // 8], F32, tag="htout")
                    dst_lo = 0
                for half in range(2):
                    nc.vector.tensor_reduce(
                        out=fc_dst[:, half, dst_lo:dst_lo + ngrp],
                        in_=f[:, half, :n].rearrange("p (g e) -> p g e", e=8),
                        axis=mybir.AxisListType.X, op=OP.add)
                    with nc.allow_low_precision("h-tilde feeds fp32r matmul"):
                        nc.vector.tensor_reduce(
                            out=ht_dst[:, half, dst_lo:dst_lo + ngrp],
                            in_=h[:, half, :n].bitcast(F32).rearrange("p (g e) -> p g e", e=8),
                            axis=mybir.AxisListType.X, op=OP.add)
                if dram_lo is not None:
                    nc.sync.dma_start(out=fc5_dram[:, dram_lo:dram_lo + ngrp].rearrange("(k p) n -> p k n", p=128),
                                      in_=fc_dst[:, :, :ngrp])
                    nc.sync.dma_start(out=ht5_dram[:, dram_lo:dram_lo + ngrp].rearrange("(k p) n -> p k n", p=128),
                                      in_=ht_dst[:, :, :ngrp])

            # ---------------- phase A: 40 slots of 512 leaf children
            G = 4
            for grp in range(FULL_SLOTS // G):
                hcs = []
                for s in range(grp * G, grp * G + G):
                    xk = load_xT(OFF_A + s * T, T)
                    hcs.append(process_head(xk, T, None, None, OFF_A + s * T))
                for i, s in enumerate(range(grp * G, grp * G + G)):
                    h_, c_ = hcs[i]
                    process_tail(h_, c_, T, 64, None, None, 0,
                                 xf_sb_slice=xfB_sb[:, :, s * 64:(s + 1) * 64],
                                 dram_lo=s * 64)

            # ---------------- phase B: 8 tiles of 512 L5 nodes
            for grp in range(2):
                hcs = []
                for t in range(grp * 4, grp * 4 + 4):
                    xk = load_xT(OFF_B + t * T, T)
                    internal = t < 5   # cols [0,2560) of ht5/fc5 are written
                    if internal:
                        ht_in = bp.tile([128, 2, T], F32R, tag="htin")
                        nc.sync.dma_start(out=ht_in[:],
                                          in_=ht5_dram[:, t * T:(t + 1) * T].rearrange("(k p) n -> p k n", p=128).bitcast(F32R))
                        fc_in = bp.tile([128, 2, T], F32, tag="fcin")
                        nc.sync.dma_start(out=fc_in[:],
                                          in_=fc5_dram[:, t * T:(t + 1) * T].rearrange("(k p) n -> p k n", p=128))
                        ht_ap, fc_ap = ht_in[:, :, :], fc_in[:, :, :]
                    else:
                        ht_ap, fc_ap = None, None
                    hcs.append(process_head(xk, T, ht_ap, fc_ap, OFF_B + t * T))
                for i, t in enumerate(range(grp * 4, grp * 4 + 4)):
                    h_, c_ = hcs[i]
                    process_tail(h_, c_, T, 64, fc4, ht4, t * 64, xf_lo=XF_C + t * 64)

            # ---------------- phase C: 512 L4 nodes
            xk = load_xT(OFF_C, T)
            h_, c_ = process_head(xk, T, ht4[:, :, :], fc4[:, :, :], OFF_C)
            process_tail(h_, c_, T, 64, fc3, ht3, 0, xf_lo=XF_D)

            # ---------------- phase D: 64 L3 nodes
            xk = load_xT(OFF_D, L3_PC)
            h_, c_ = process_head(xk, L3_PC, ht3[:, :, :], fc3[:, :, :], OFF_D)
            process_tail(h_, c_, L3_PC, 8, fc2, ht2, 0, xf_lo=XF_E)

            # ---------------- phase E: 8 L2 nodes; emit h and c, done
            xk = load_xT(OFF_E, L2_PC)
            h2t, c2t = process_head(xk, L2_PC, ht2[:, :, :], fc2[:, :, :], OFF_E)
            nc.sync.dma_start(out=hAll[:, OFF_CE:OFF_CE + L2_PC].rearrange("(k p) n -> p k n", p=128),
                              in_=c2t[:, :, :L2_PC])

    nc.compile()
    return nc


def _get_nc(variant="main"):
    key = "nc_" + variant
    if key not in _STATE:
        _STATE[key] = _build_program(null=(variant == "null"))
    return _STATE[key]


# ---------------------------------------------------------------- host glue
def _pack_inputs(x, W_iou, U_iou, W_f, U_f):
    maps = _index_maps()
    x = np.asarray(x, dtype=np.float32)
    x_pad = np.vstack([x, np.zeros((1, D), np.float32)])  # row N = zeros (for -1 ids)
    w_block = np.concatenate([
        np.asarray(W_iou, np.float32), np.asarray(U_iou, np.float32),
        np.asarray(W_f, np.float32), np.asarray(U_f, np.float32)], axis=1)
    in_maps = []
    for c in range(N_CORES):
        m = maps[c]
        xsec = np.concatenate([x_pad[m[name]].T for name in ("A", "B", "C", "D", "E")],
                              axis=1)
        in_maps.append({"xw": np.ascontiguousarray(
            np.concatenate([xsec, w_block], axis=1))})
    return in_maps


def _sig(z):
    return 1.0 / (1.0 + np.exp(-z))


def _host_top(x, W_iou, U_iou, b_iou, W_f, U_f, b_f, hL2, cL2):
    """Finish L1 nodes 1..8 and the root on the host from L2 (h, c).

    hL2/cL2 are [64, D] for global nodes 9..72; children of L1 node p are
    nodes 8p+1..8p+8, i.e. L2 block p-1."""
    hc = hL2.reshape(8, 8, D)
    cc = cL2.reshape(8, 8, D)
    hL1 = np.zeros((8, D), np.float32)
    cL1 = np.zeros((8, D), np.float32)
    out = np.zeros((9, D), np.float32)

    def node(xrow, ch_h, ch_c):
        h_t = ch_h.sum(0)
        f = _sig(xrow @ W_f + b_f + ch_h @ U_f)
        fc = (f * ch_c).sum(0)
        iou = xrow @ W_iou + b_iou + h_t @ U_iou
        cg = _sig(iou[:D]) * np.tanh(iou[2 * D:]) + fc
        hg = _sig(iou[D:2 * D]) * np.tanh(cg)
        return hg, cg

    for p in range(1, 9):
        hL1[p - 1], cL1[p - 1] = node(x[p], hc[p - 1], cc[p - 1])
        out[p] = hL1[p - 1]
    out[0], _ = node(x[0], hL1, cL1)
    return out


def _unpack_outputs(results, x, W_iou, U_iou, b_iou, W_f, U_f, b_f):
    maps = _index_maps()
    out = np.empty((N, D), dtype=np.float32)
    sections = (("A", OFF_A, A_COLS), ("B", OFF_B, L5_PC), ("C", OFF_C, L4_PC),
                ("D", OFF_D, L3_PC), ("E", OFF_E, L2_PC))
    hL2 = np.empty((64, D), np.float32)
    cL2 = np.empty((64, D), np.float32)
    for c in range(N_CORES):
        m = maps[c]
        hT = results[c]["hAll"].T          # [H_COLS, D]
        for name, lo, ncols in sections:
            ids = m[name]
            valid = ids >= 0
            out[ids[valid]] = hT[lo:lo + ncols][valid]
        for k in range(8):                 # L2 node S[2]+c+8k -> index c+8k
            hL2[c + 8 * k] = hT[OFF_E + k]
            cL2[c + 8 * k] = hT[OFF_CE + k]
    out[0:9] = _host_top(np.asarray(x, np.float32)[0:9],
                         np.asarray(W_iou, np.float32), np.asarray(U_iou, np.float32),
                         np.asarray(b_iou, np.float32),
                         np.asarray(W_f, np.float32), np.asarray(U_f, np.float32),
                         np.asarray(b_f, np.float32), hL2, cL2)
    return out


# ---------------------------------------------------------------- jitted runner
def _get_runner(variant="main"):
    """Compile the SPMD executable once (fast-dispatch); returns helpers."""
    rkey = "runner_" + variant
    if rkey in _STATE:
        return _STATE[rkey]
    import jax
    import jax.numpy as jnp
    from jax.sharding import Mesh, PartitionSpec, NamedSharding
    from jax.experimental.shard_map import shard_map
    import concourse.mybir as mybir
    from concourse import bass2jax

    nc = _get_nc(variant)
    bass2jax.install_neuronx_cc_hook()

    partition_name = nc.partition_id_tensor.name if nc.partition_id_tensor else None
    in_names, out_names, out_avals = [], [], []
    for alloc in nc.m.functions[0].allocations:
        if not isinstance(alloc, mybir.MemoryLocationSet):
            continue
        name = alloc.memorylocations[0].name
        if alloc.kind == "ExternalInput":
            if name != partition_name:
                in_names.append(name)
        elif alloc.kind == "ExternalOutput":
            out_names.append(name)
            out_avals.append(jax.core.ShapedArray(tuple(alloc.tensor_shape),
                                                  mybir.dt.np(alloc.dtype)))
    n_params = len(in_names)
    n_outs = len(out_names)
    all_names = in_names + out_names
    if partition_name is not None:
        all_names = all_names + [partition_name]

    def _body(*args):
        operands = list(args)
        if partition_name is not None:
            operands.append(bass2jax.partition_id_tensor())
        outs = bass2jax._bass_exec_p.bind(
            *operands,
            out_avals=tuple(out_avals),
            in_names=tuple(all_names),
            out_names=tuple(out_names),
            lowering_input_output_aliases=(),
            sim_require_finite=True,
            sim_require_nnan=True,
            nc=nc,
        )
        return tuple(outs)

    devices = jax.devices()[:N_CORES]
    mesh = Mesh(np.asarray(devices), ("core",))
    sharding = NamedSharding(mesh, PartitionSpec("core"))
    donate = tuple(range(n_params, n_params + n_outs))
    state = {}

    def _fast(args):
        if "c" not in state:
            def compile_fn():
                jitted = jax.jit(
                    shard_map(_body, mesh=mesh,
                              in_specs=(PartitionSpec("core"),) * (n_params + n_outs),
                              out_specs=(PartitionSpec("core"),) * n_outs,
                              check_rep=False),
                    donate_argnums=donate, keep_unused=True)
                return jitted.lower(*args).compile()
            state["c"] = bass2jax.fast_dispatch_compile(compile_fn)
        return state["c"]

    def make_zero_outs():
        outs = []
        for a in out_avals:
            shape = (N_CORES * a.shape[0], *a.shape[1:])
            outs.append(jax.jit(lambda s=shape, d=a.dtype: jnp.zeros(s, d),
                                out_shardings=sharding)())
        return outs

    def put_inputs(in_maps):
        return [jax.device_put(
            np.concatenate([np.asarray(in_maps[c][n]) for c in range(N_CORES)], axis=0),
            sharding) for n in in_names]

    def run_nosync(dev_inputs, zero_outs):
        return _fast((*dev_inputs, *zero_outs))(*dev_inputs, *zero_outs)

    def run(dev_inputs, zero_outs):
        out_arrs = run_nosync(dev_inputs, zero_outs)
        jax.block_until_ready(out_arrs)
        return out_arrs

    def to_results(out_arrs):
        return [
            {name: np.asarray(out_arrs[i]).reshape(N_CORES, *out_avals[i].shape)[c]
             for i, name in enumerate(out_names)}
            for c in range(N_CORES)
        ]

    r = dict(run=run, run_nosync=run_nosync, put_inputs=put_inputs,
             make_zero_outs=make_zero_outs,
             to_results=to_results, out_names=out_names, in_names=in_names)
    _STATE[rkey] = r
    return r


def _run(in_maps):
    r = _get_runner()
    dev_inputs = r["put_inputs"](in_maps)
    out_arrs = r["run"](dev_inputs, r["make_zero_outs"]())
    return r["to_results"](out_arrs)


def kernel(x, W_iou, U_iou, b_iou, W_f, U_f, b_f, parent_idx, level, num_levels):
    b_iou = np.asarray(b_iou)
    b_f = np.asarray(b_f)
    parent_idx = np.asarray(parent_idx)
    expect_parent = (np.arange(1, N, dtype=np.int64) - 1) // BR
    assert parent_idx.shape == (N - 1,) and np.array_equal(parent_idx, expect_parent), \
        "kernel is specialized to the deterministic 8-ary tree parent(i)=(i-1)//8"
    assert not b_iou.any() and not b_f.any(), "device program assumes zero biases"
    in_maps = _pack_inputs(x, W_iou, U_iou, W_f, U_f)
    results = _run(in_maps)
    return _unpack_outputs(results, x, W_iou, U_iou, b_iou, W_f, U_f, b_f)
